# revision 16
# baseline (speedup 1.0000x reference)
"""KANLayer kernel for 8 Trainium2 NeuronCores (raw Bass, explicit semaphores).

Reference computation (B=4096, D=1024, O=1024, S=4 spline points):
    xmin/xmax = per-feature min/max of x over the batch dim      # [1, D]
    xn  = (x - xmin) / (xmax - xmin)                             # [B, D]
    c   = spline_coeffs.sum(axis=2)                              # [O, D, 4]
    out = xn^3 @ c0.T + xn^2 @ c1.T + xn @ c2.T + c3.sum(d)     # [B, O]

Sharding: tensor-parallel over the output dim O; core r owns output columns
[128r, 128r+128). Every core loads the full xT [D, B] (the contraction runs
over all D features) and computes the per-feature batch min/max for ALL
features locally — there is NO collective and no cross-core dependency, so
a core's execution time is independent of the other cores' launch skew
(the previous kernel's AllGather made every core wait for the slowest
core's launch).

Local stats would cost 2x 4.3us/chunk as direct f32 DVE reduces (DVE is
the only engine that can reduce along the free axis, and tensor_reduce has
no fast mode). Instead the min/max runs as a binary fold tree in bf16:
fold1 reads the f32 tile and writes bf16 (1x), folds 2-4 are pure bf16
tensor_tensor min/max which the DVE runs at 2 elem/lane/cycle (2x_1p), and
a final 256-wide reduce produces [P, 1]. Rounding to bf16 is monotone, so
max(round(x)) == round(max(x)): the stat error is just the bf16 rounding
of the true min/max (~2^-9 relative), far inside the 2e-2 gate.

x^3 = xn * xn^2 needs a tensor*tensor multiply (DVE-only). To balance
engines, columns [0, X3W) are multiplied on DVE (scalar_tensor_tensor) and
columns [X3W, B) are computed on ACT as exp(3*ln(xn + 1e-30)) — Ln, Exp,
Relu, Square and Copy all live in the same ACT table set
(natural_log_exp_and_others), so there is no table-reload cost.

The constant term sum_d c3[o,d] is folded into the PE accumulation: after
the 8 d-chunks, one extra matmul per PSUM bank with lhsT = c3j [d, o]
(= sum_j of the k=3 coefficient plane, reduced on DVE) and rhs = an
all-ones [128, 512] f32r tile adds the bias to every batch column. PSUM
banks then drain via ACT Copy into a 2-slot staging ring and DMA out.
HBM traffic per core: xt 16MB + coeffs 8MB + out 2MB = 26MB (the baseline
moved 30MB).

Spline-coefficient prep runs on the DMA engines: the host supplies the
shard as [S, 4, D, 128]; two parallel 2-deep SWDGE accumulate chains
(copy + accum_op=add) pair-sum the spline planes, and one DVE
scalar_tensor_tensor merges the pairs while rounding fp32 -> float32r
(walrus requires f32r matmul operands be *written* as f32r).

Matmuls run in float32r (fp32 bits, FP22 truncation inside the PE): 1 PE
cycle/row at N=512 with ~2^-14 input rounding.

Toolchain constraints honored here:
  * walrus lowers at most ONE semaphore wait per instruction -> every wait
    is a standalone wait_ge;
  * the sim race detector does not credit same-engine program order, so
    intra-engine data deps carry explicit self-sem chains (s_dv);
  * a DMA's then_inc(sem, 16) lands as 16 separate +1s, so concurrently
    in-flight DMAs use different semaphores (s_xte/s_xto parity split);
  * memset cannot write f32r and the Pool engine (gpsimd) has no native
    elementwise ISA ops, so the ones tile is staged via a bf16 memset +
    DVE convert, and Pool only issues DMAs.

n_iters > 1 builds a NEFF that runs the whole kernel N times back-to-back
(for device-time measurement by wall-clock slope; the axon tunnel's
per-call input shipping makes single-run wall time meaningless).

Output per core is out_t [128, B] (transposed); the host concatenates the
8 shards and transposes back.
"""

import numpy as np

import concourse.bass as bass
import concourse.mybir as mybir
from concourse.bass_utils import run_bass_kernel_spmd

P = 128            # SBUF partitions / rows per tile
B = 4096           # batch
D = 1024           # input features
O = 1024           # output features
S = 4              # spline points
KC = 4             # cubic coefficients per (o, d)
NCORES = 8
OS = O // NCORES   # output columns per core = 128
DC = D // P        # d-chunks = 8
QW = 512           # matmul moving-dim width (one PSUM bank)
NQ = B // QW       # 8

X3W = 2304         # x3 columns multiplied on DVE; ACT exp/ln does the rest
# bf16 fold scratch regions (per stat, per slot), units = bf16 elements:
#   fold1 (f32 -> bf16)  [0:2048)
#   fold2 (bf16)         [2048:3072)
#   fold3 (bf16)         [0:512)
#   fold4 (bf16)         [512:768)   <- final 256-wide reduce reads this
GW = 3072

F32 = mybir.dt.float32
F32R = mybir.dt.float32r
BF16 = mybir.dt.bfloat16
AX = mybir.AxisListType
ALU = mybir.AluOpType
ACTF = mybir.ActivationFunctionType

_CACHE = {}


def _pe_tick(g: int) -> int:
    """s_pe value after chunk g's matmuls retired (9 ticks/iter: 8 chunks
    + 1 for the bias matmuls)."""
    return 9 * (g // 8) + (g % 8) + 1


def _build_bass(n_iters: int = 1, timing_mode: bool = False) -> bass.Bass:
    nc = bass.Bass(num_devices=NCORES)

    kind = {} if timing_mode else {"kind": "ExternalInput"}
    okind = {} if timing_mode else {"kind": "ExternalOutput"}
    xt = nc.dram_tensor("xt", [D, B], F32, **kind)
    # [S, KC, D, OS]: s-major so each spline plane is one contiguous DMA
    coeffs = nc.dram_tensor("coeffs", [S, KC, D, OS], F32, **kind)
    out_t = nc.dram_tensor("out_t", [OS, B], F32, **okind)
    dummy = (
        nc.dram_tensor("tout", [P, 2], F32, kind="ExternalOutput")
        if timing_mode
        else None
    )

    from contextlib import ExitStack

    ctx = ExitStack()
    with ctx:
        sem = lambda name: ctx.enter_context(nc.semaphore(name))  # noqa: E731
        s_xte = sem("s_xte")      # +16 per even-chunk xt load (slot 0)
        s_xto = sem("s_xto")      # +16 per odd-chunk xt load (slot 1)
        s_cfa = sem("s_cfa")      # +16 per coeff plane DMA (pair A: s0, s1)
        s_cfb = sem("s_cfb")      # +16 per coeff plane DMA (pair B: s2, s3)
        s_stats = sem("s_stats")  # +1 per chunk: scale/bias ready
        s_act = sem("s_act")      # +1 per ACT op (xn, x2, ln, exp): 32/iter
        s_x3 = sem("s_x3")        # +1 per chunk: DVE x3 head done
        s_dv = sem("s_dv")        # DVE same-engine retirement chain
        s_pe = sem("s_pe")        # 9/iter: 8 chunk matmul groups + bias
        s_conv = sem("s_conv")    # +1/iter: c_allr merged
        s_c3j = sem("s_c3j")      # +1/iter: c3j (bias lhsT) ready
        s_out = sem("s_out")      # +16 per output DMA: 128/iter
        s_dr = sem("s_dr")        # +1 per ACT psum->staging drain: 8/iter
        s_one = sem("s_one")      # +2 once: ones tile init
        s_fin = sem("s_fin")      # timing_mode DRAM init / dummy output

        sb = lambda name, shape, dtype=F32: ctx.enter_context(  # noqa: E731
            nc.sbuf_tensor(name, shape, dtype)
        )
        xt_sb = [sb(f"xt{i}", [P, B]) for i in range(2)]
        xn_sb = [sb(f"xn{i}", [P, B], F32R) for i in range(2)]
        x2_sb = [sb(f"x2{i}", [P, B], F32R) for i in range(2)]
        x3_sb = [sb(f"x3{i}", [P, B], F32R) for i in range(2)]
        cpa = sb("cpa", [P, KC, DC, OS])
        cpb = sb("cpb", [P, KC, DC, OS])
        c_allr = sb("c_allr", [P, KC, DC, OS], F32R)
        c3j = sb("c3j", [P, OS], F32R)
        ones = sb("ones", [P, QW], F32R)
        gmn = sb("gmn", [P, 2, GW], BF16)
        gmx = sb("gmx", [P, 2, GW], BF16)
        ostg = sb("ostg", [P, 2, QW])  # psum->DRAM staging ring
        mn_sb = sb("mn", [P, 2])
        mx_sb = sb("mx", [P, 2])
        rr_sb = sb("rr", [P, 2])
        st_sb = sb("st", [P, 2, 2])  # [:, slot, 0]=scale, [:, slot, 1]=bias

        psum = ctx.enter_context(nc.psum_tensor("ps", [P, B], F32))

        NI = n_iters

        def ld_sem(j):
            return s_xte if j % 2 == 0 else s_xto

        def ld_cnt(it, j):
            return 16 * (4 * it + j // 2 + 1)

        # s_dv ops per chunk: 8 folds + 2 final reduces (+ rr, s in real
        # mode; t / the timing-mode memset increments s_stats instead)
        DVC = 10 if timing_mode else 12

        with nc.Block() as block:

            @block.sync
            def _(sp):
                if timing_mode:
                    sp.wait_ge(s_fin, 1)  # xt_sb[0] memset by DVE
                    zsrc = xt_sb[0][:, :]
                    sp.dma_start(
                        out=xt[:, :].rearrange("(n p) f -> p n f", p=P),
                        in_=bass.AP(
                            tensor=zsrc.tensor,
                            offset=zsrc.offset,
                            ap=[[zsrc.ap[0][0], P], [0, D // P], [1, B]],
                        ),
                    ).then_inc(s_fin, 16)
                    sp.wait_ge(s_fin, 17)
                    nflat = S * KC * D * OS // P  # 16384 per partition
                    sp.dma_start(
                        out=coeffs[:, :, :, :]
                        .rearrange("s k d o -> (s k d o)")
                        .rearrange("(p f) -> p f", p=P)
                        .rearrange("p (m f) -> p m f", f=B),
                        in_=bass.AP(
                            tensor=zsrc.tensor,
                            offset=zsrc.offset,
                            ap=[[zsrc.ap[0][0], P], [0, nflat // B], [1, B]],
                        ),
                    ).then_inc(s_fin, 16)
                    sp.wait_ge(s_fin, 33)
                for it in range(NI):
                    for j in range(DC):
                        g = 8 * it + j
                        if g >= 2:
                            # xt slot consumers of chunk g-2: ACT xn, DVE
                            # fold1 (covered transitively by s_stats)
                            sp.wait_ge(s_act, 4 * (g - 2) + 1)
                            sp.wait_ge(s_stats, g - 1)
                        sp.dma_start(
                            out=xt_sb[j % 2][:, :],
                            in_=xt[j * P : (j + 1) * P, :],
                        ).then_inc(ld_sem(j), 16)
                    # output: staging ring -> DRAM
                    for q in range(NQ):
                        sp.wait_ge(s_dr, 8 * it + q + 1)
                        sp.dma_start(
                            out=out_t[:, q * QW : (q + 1) * QW],
                            in_=ostg[:, q % 2, :],
                        ).then_inc(s_out, 16)
                sp.wait_ge(s_out, 128 * NI)
                if dummy is not None:
                    sp.dma_start(out=dummy[:, :], in_=st_sb[:, 0, :]).then_inc(
                        s_fin, 16
                    )
                    sp.wait_ge(s_fin, 49)

            @block.scalar
            def _(act):
                for it in range(NI):
                    for j in range(DC):
                        g = 8 * it + j
                        sl = j % 2
                        act.wait_ge(s_stats, g + 1)
                        if g >= 2:
                            # xn/x2/x3 slot recycle: PE + DVE readers of g-2
                            act.wait_ge(s_pe, _pe_tick(g - 2))
                            act.wait_ge(s_x3, g - 1)
                            act.wait_ge(s_act, 4 * (g - 2) + 4)
                        act.activation(
                            xn_sb[sl][:, :],
                            xt_sb[sl][:, :],
                            ACTF.Relu,
                            bias=st_sb[:, sl, 1:2],
                            scale=st_sb[:, sl, 0:1],
                        ).then_inc(s_act)
                        act.wait_ge(s_act, 4 * g + 1)  # xn retired
                        act.activation(
                            x2_sb[sl][:, :],
                            xn_sb[sl][:, :],
                            ACTF.Square,
                        ).then_inc(s_act)
                        # x3 tail = exp(3*ln(xn + 1e-30)); ln lands in the
                        # x3 buffer, exp rewrites it in place
                        act.wait_ge(s_act, 4 * g + 2)
                        # ln(0) = -inf is fine: exp(3*-inf) = 0 = 0^3
                        act.activation(
                            x3_sb[sl][:, X3W:],
                            xn_sb[sl][:, X3W:],
                            ACTF.Ln,
                        ).then_inc(s_act)
                        act.wait_ge(s_act, 4 * g + 3)
                        act.activation(
                            x3_sb[sl][:, X3W:],
                            x3_sb[sl][:, X3W:],
                            ACTF.Exp,
                            scale=3.0,
                        ).then_inc(s_act)
                    # psum -> staging drains (Copy adds nothing)
                    act.wait_ge(s_pe, 9 * (it + 1))
                    for q in range(NQ):
                        if 8 * it + q >= 2:
                            # staging slot reused by out-DMA q-2
                            act.wait_ge(s_out, 16 * (8 * it + q - 1))
                        act.activation(
                            ostg[:, q % 2, :],
                            psum[:, q * QW : (q + 1) * QW],
                            ACTF.Copy,
                        ).then_inc(s_dr)

            @block.vector
            def _(dve):
                if timing_mode:
                    dve.memset(xt_sb[0][:, :], 0.3).then_inc(s_fin)
                # ones tile: memset can't write f32r; stage 1.0 in the
                # (not-yet-used) bf16 fold scratch and convert via +0.0
                dve.memset(gmn[:, 0, 0:QW], 1.0).then_inc(s_one)
                dve.wait_ge(s_one, 1)
                dve.tensor_scalar_add(
                    ones[:, :], gmn[:, 0, 0:QW], 0.0
                ).then_inc(s_one)

                def x3_head(g):
                    sl = g % 2
                    dve.wait_ge(s_act, 4 * g + 2)  # x2(g) ready
                    if g >= 2:
                        dve.wait_ge(s_pe, _pe_tick(g - 2))  # x3 slot free
                    dve.scalar_tensor_tensor(
                        x3_sb[sl][:, :X3W],
                        x2_sb[sl][:, :X3W],
                        1.0,
                        xn_sb[sl][:, :X3W],
                        ALU.bypass,
                        ALU.mult,
                    ).then_inc(s_x3)

                for it in range(NI):
                    for j in range(DC):
                        g = 8 * it + j
                        sl = j % 2
                        c0 = DVC * g
                        dve.wait_ge(ld_sem(j), ld_cnt(it, j))
                        if g == 0:
                            # fold scratch doubles as ones staging
                            dve.wait_ge(s_one, 2)
                        if g >= 2:
                            # scratch + mn/mx/st slot recycle: chunk g-2's
                            # full stat chain retired
                            dve.wait_ge(s_stats, g - 1)
                        # fold tree: min and max interleaved so each op's
                        # self-dep is two instructions back (no stall)
                        for t_, buf, op in ((0, gmn, ALU.min), (1, gmx, ALU.max)):
                            dve.tensor_tensor(
                                buf[:, sl, 0:2048],
                                xt_sb[sl][:, 0:2048],
                                xt_sb[sl][:, 2048:B],
                                op,
                            ).then_inc(s_dv)
                        for t_, buf, op in ((0, gmn, ALU.min), (1, gmx, ALU.max)):
                            dve.wait_ge(s_dv, c0 + 1 + t_)
                            dve.tensor_tensor(
                                buf[:, sl, 2048:3072],
                                buf[:, sl, 0:1024],
                                buf[:, sl, 1024:2048],
                                op,
                            ).then_inc(s_dv)
                        for t_, buf, op in ((0, gmn, ALU.min), (1, gmx, ALU.max)):
                            dve.wait_ge(s_dv, c0 + 3 + t_)
                            dve.tensor_tensor(
                                buf[:, sl, 0:512],
                                buf[:, sl, 2048:2560],
                                buf[:, sl, 2560:3072],
                                op,
                            ).then_inc(s_dv)
                        for t_, buf, op in ((0, gmn, ALU.min), (1, gmx, ALU.max)):
                            dve.wait_ge(s_dv, c0 + 5 + t_)
                            dve.tensor_tensor(
                                buf[:, sl, 512:768],
                                buf[:, sl, 0:256],
                                buf[:, sl, 256:512],
                                op,
                            ).then_inc(s_dv)
                        for t_, buf, op, out in (
                            (0, gmn, ALU.min, mn_sb),
                            (1, gmx, ALU.max, mx_sb),
                        ):
                            dve.wait_ge(s_dv, c0 + 7 + t_)
                            dve.tensor_reduce(
                                out[:, sl : sl + 1],
                                buf[:, sl, 512:768],
                                axis=AX.X,
                                op=op,
                            ).then_inc(s_dv)
                        if g >= 2:
                            # st slot WAR vs ACT xn(g-2) scale/bias read
                            dve.wait_ge(s_act, 4 * (g - 2) + 1)
                        if timing_mode:
                            dve.wait_ge(s_dv, c0 + 10)  # finals retired
                            dve.memset(st_sb[:, sl, :], 0.25).then_inc(
                                s_stats
                            )
                        else:
                            dve.wait_ge(s_dv, c0 + 10)
                            dve.tensor_sub(
                                rr_sb[:, sl : sl + 1],
                                mx_sb[:, sl : sl + 1],
                                mn_sb[:, sl : sl + 1],
                            ).then_inc(s_dv)
                            dve.wait_ge(s_dv, c0 + 11)
                            dve.reciprocal(
                                st_sb[:, sl, 0:1], rr_sb[:, sl : sl + 1]
                            ).then_inc(s_dv)
                            dve.wait_ge(s_dv, c0 + 12)
                            # t = (mn * -1) * s
                            dve.scalar_tensor_tensor(
                                st_sb[:, sl, 1:2],
                                mn_sb[:, sl : sl + 1],
                                -1.0,
                                st_sb[:, sl, 0:1],
                                ALU.mult,
                                ALU.mult,
                            ).then_inc(s_stats)
                        # one-stage pipeline: emit chunk g-1's x3 head here
                        if j >= 1:
                            x3_head(g - 1)
                        if j == 2:
                            # coefficient merge: pair-sum -> f32r
                            dve.wait_ge(s_cfa, 32 * it + 32)
                            dve.wait_ge(s_cfb, 32 * it + 32)
                            if it > 0:
                                # c_allr/c3j WAR vs prev iter's matmuls
                                dve.wait_ge(s_pe, 9 * it)
                            dve.scalar_tensor_tensor(
                                c_allr[:, :, :, :],
                                cpa[:, :, :, :],
                                1.0,
                                cpb[:, :, :, :],
                                ALU.bypass,
                                ALU.add,
                            ).then_inc(s_conv)
                            dve.wait_ge(s_conv, it + 1)
                            with nc.allow_low_precision(
                                "float32r is fp32-width storage"
                            ):
                                dve.tensor_reduce(
                                    c3j[:, :],
                                    c_allr[:, 3, :, :].rearrange(
                                        "p j o -> p o j"
                                    ),
                                    axis=AX.X,
                                    op=ALU.add,
                                ).then_inc(s_c3j)
                    x3_head(8 * it + 7)

            @block.tensor
            def _(pe):
                pe.wait_ge(s_one, 2)
                for it in range(NI):
                    if it > 0:
                        pe.wait_ge(s_out, 128 * it)  # psum drained
                    pe.wait_ge(s_conv, it + 1)
                    for j in range(DC):
                        g = 8 * it + j
                        sl = j % 2
                        pe.wait_ge(s_x3, g + 1)
                        pe.wait_ge(s_act, 4 * g + 4)  # x3 tail (exp) done
                        for k in range(3):  # 0: c0*x3, 1: c1*x2, 2: c2*xn
                            src = [x3_sb, x2_sb, xn_sb][k][sl]
                            for q in range(NQ):
                                mm = pe.matmul(
                                    psum[:, q * QW : (q + 1) * QW],
                                    lhsT=c_allr[:, k, j, :],
                                    rhs=src[:, q * QW : (q + 1) * QW],
                                    start=(j == 0 and k == 0),
                                    stop=False,
                                )
                        mm.then_inc(s_pe)
                    pe.wait_ge(s_c3j, it + 1)
                    for q in range(NQ):
                        mm = pe.matmul(
                            psum[:, q * QW : (q + 1) * QW],
                            lhsT=c3j[:, :],
                            rhs=ones[:, :],
                            start=False,
                            stop=True,
                        )
                    mm.then_inc(s_pe)

            @block.gpsimd
            def _(pool):
                if timing_mode:
                    pool.wait_ge(s_fin, 33)  # coeffs DRAM initialized
                for it in range(NI):
                    if it > 0:
                        pool.wait_ge(s_conv, it)  # cpa/cpb WAR vs merge
                    pool.dma_start(
                        out=cpa[:, :, :, :],
                        in_=coeffs[0].rearrange("k (j p) o -> p k j o", p=P),
                    ).then_inc(s_cfa, 16)
                    pool.dma_start(
                        out=cpb[:, :, :, :],
                        in_=coeffs[2].rearrange("k (j p) o -> p k j o", p=P),
                    ).then_inc(s_cfb, 16)
                    pool.wait_ge(s_cfa, 32 * it + 16)
                    pool.dma_start(
                        out=cpa[:, :, :, :],
                        in_=coeffs[1].rearrange("k (j p) o -> p k j o", p=P),
                        accum_op=ALU.add,
                    ).then_inc(s_cfa, 16)
                    pool.wait_ge(s_cfb, 32 * it + 16)
                    pool.dma_start(
                        out=cpb[:, :, :, :],
                        in_=coeffs[3].rearrange("k (j p) o -> p k j o", p=P),
                        accum_op=ALU.add,
                    ).then_inc(s_cfb, 16)

    return nc


def get_bass(n_iters: int = 1, timing_mode: bool = False) -> bass.Bass:
    key = f"nc{n_iters}_{timing_mode}"
    if key not in _CACHE:
        _CACHE[key] = _build_bass(n_iters, timing_mode)
    return _CACHE[key]


def make_in_maps(x: np.ndarray, spline_coeffs: np.ndarray):
    """Host-side sharding/marshaling only (slicing + transposes, no math)."""
    x = np.ascontiguousarray(np.asarray(x, dtype=np.float32))
    spline_coeffs = np.ascontiguousarray(np.asarray(spline_coeffs, dtype=np.float32))
    xt = np.ascontiguousarray(x.T)  # [D, B]
    in_maps = []
    for r in range(NCORES):
        shard = spline_coeffs[r * OS : (r + 1) * OS]  # [OS, D, S, KC]
        in_maps.append(
            {
                "xt": xt,
                # [S, KC, D, OS]
                "coeffs": np.ascontiguousarray(shard.transpose(2, 3, 1, 0)),
            }
        )
    return in_maps


def assemble_output(results) -> np.ndarray:
    out = np.concatenate([results[r]["out_t"] for r in range(NCORES)], axis=0)
    return np.ascontiguousarray(out.T)  # [B, O]


def run(x: np.ndarray, spline_coeffs: np.ndarray, trace: bool = False,
        n_iters: int = 1):
    """Returns (output, BassKernelResults)."""
    nc = get_bass(n_iters)
    in_maps = make_in_maps(x, spline_coeffs)
    res = run_bass_kernel_spmd(nc, in_maps, list(range(NCORES)), trace=trace)
    return assemble_output(res.results), res


def kernel(x: np.ndarray, spline_coeffs: np.ndarray) -> np.ndarray:
    out, _ = run(x, spline_coeffs, trace=False)
    return out
